# revision 14
# baseline (speedup 1.0000x reference)
"""Single-head causal attention on 8 TRN2 NeuronCores.

Problem: x[B=8, T=2048, C=1024], Wq/Wk/Wv[C, H=64] (fp32)
  q = x@Wq; k = x@Wk; v = x@Wv
  wei = softmax(mask(q k^T * C^-0.5)); out = wei @ v       -> [B, T, H]

Sharding: data-parallel over batch, one batch element per core.

Per-core dataflow (all matmuls bf16, fp32 PSUM accumulation):
  1. x [T,C] fp32 --SWDGE cast DMA--> x_nat bf16 [128, 16, 1024] (t-tiles)
  2. xbar transpose DMAs -> xT bf16 [128, 8, 2048]  (C on partitions)
  3. QKV: packed [Wq|Wk] stationary -> psum [qT;kT], Wv -> vT
     scale 1/sqrt(C) is folded into the exp() activation.
  4. S^T tiles = kT.T @ qT  (keys on partitions); causal mask added on
     diagonal tiles; exp on ScalarE (no max subtraction needed: logits
     are O(1) by construction); PV: out_un^T[65, T] accumulates
     [v|ones].T @ exp(S^T)  -- row 64 = sumexp for free.
  5. PE-transpose out_un^T chunks, multiply by 1/sumexp, DMA out.
"""
import sys

sys.path.insert(0, "/opt/trn_rl_repo")

import numpy as np

import concourse.bass as bass
import concourse.mybir as mybir
import concourse.tile as tile
from concourse import bacc
from concourse.bass_utils import run_bass_kernel_spmd
from concourse.masks import make_identity

B, T, C, H = 8, 2048, 1024, 64
NTT = T // 128   # 16 t-tiles
NCT = C // 128   # 8  c-tiles
NCH = T // 512   # 4  t-chunks (moving free dim)
SCALE = float(C) ** -0.5
MASKVAL = -32768.0  # pre-scale additive mask; * SCALE -> -1024 -> exp -> 0
VP = 80          # v_nat per-tile stride: 160B, 32B-aligned for xbar transpose

F32 = mybir.dt.float32
BF16 = mybir.dt.bfloat16

DEBUG_TAPS = False


def emit_body(nc, tc, xD, outD, consts, pools):
    AF = mybir.ActivationFunctionType
    ALU = mybir.AluOpType
    wqk, wv, maskd, ident = consts
    xnpool, xtpool, qkpool, ptpool, opool, fpool = pools

    # ---- load x (cast fp32->bf16) and transpose to xT ----
    x_nat = xnpool.tile([128, NTT, C], BF16, tag="xnat")
    xt = xtpool.tile([128, NCT, T], BF16, tag="xt")
    for tk in range(NTT):
        nc.gpsimd.dma_start(x_nat[:, tk, :], xD[tk * 128:(tk + 1) * 128, :])
        # one multi-tile xbar transpose per t-tile: [128t, 1024c] ->
        # out[c_lo, c_hi, t] scattered across the 8 c-tiles of xT.
        # Alternate the two HWDGE engines so transposes overlap.
        eng = nc.sync if tk % 2 == 0 else nc.scalar
        eng.dma_start(
            xt[:, :, tk * 128:(tk + 1) * 128],
            x_nat[:, tk, :],
            transpose=True,
        )

    # ---- QKV projections + attention, pipelined per 512-wide t-chunk ----
    qk_a = qkpool.tile([128, T], BF16, tag="qka")   # rows 0:64 qT, 64:128 kT
    kt_lo = qkpool.tile([64, T], BF16, tag="ktlo")  # kT at partitions 0:64
    vt = qkpool.tile([64, T], BF16, tag="vt")       # vT at partitions 0:64
    v_nat = qkpool.tile([128, NTT, VP], BF16, tag="vnat")  # [s_lo, s_hi, v|1]
    nc.gpsimd.memset(v_nat[:, :, H:H + 1], 1.0)
    o_out = fpool.tile([128, NTT, H], F32, tag="oout")
    outR = outD.rearrange("(g p) h -> p g h", p=128)
    with (
        tc.tile_pool(name="qkps", bufs=2, space="PSUM") as qkps,
        tc.tile_pool(name="vps", bufs=1, space="PSUM") as vps,
        tc.tile_pool(name="ops", bufs=2, space="PSUM") as ops,
        tc.tile_pool(name="stps", bufs=2, space="PSUM") as stps,
        tc.tile_pool(name="fps", bufs=1, space="PSUM") as fps,
    ):
        # PE warm-up: dummy matmuls gated on the first transpose so the
        # HAM clock-gate reaches 8/8 before (and during) real QKV work.
        warm = qkps.tile([128, 512], F32, tag="psqk")
        for _ in range(16):
            nc.tensor.matmul(
                warm[:], wqk[:, 0, :], xt[:, 0:4, 0:128],
                start=True, stop=True,
            )
        for n in range(NCH):
            sl = slice(n * 512, (n + 1) * 512)
            ps_v = vps.tile([64, 512], F32, tag="psv")
            for k in range(NCT):
                nc.tensor.matmul(
                    ps_v[:], wv[:, k, :], xt[:, k, sl],
                    start=(k == 0), stop=(k == NCT - 1),
                )
            nc.vector.tensor_copy(vt[:, sl], ps_v[:])
            # per-chunk xbar transpose: vT[64h, 512s] -> v_nat[s_lo, tk, h]
            nc.sync.dma_start(
                v_nat[:, n * 4:(n + 1) * 4, 0:H], vt[:, sl], transpose=True
            )
            ps_qk = qkps.tile([128, 512], F32, tag="psqk")
            for k in range(NCT):
                nc.tensor.matmul(
                    ps_qk[:], wqk[:, k, :], xt[:, k, sl],
                    start=(k == 0), stop=(k == NCT - 1),
                )
            nc.vector.tensor_copy(qk_a[:, sl], ps_qk[:])
            # kT shifted to partitions 0:64 (stationary operand of S^T)
            nc.gpsimd.dma_start(kt_lo[:, sl], qk_a[64:128, sl])

        # attention, column-chunk outer: chunk ci only needs QKV chunks
        # <= ci, so early chunks overlap later projections
        for ci in range(NCH):
            out_pc = ops.tile([H + 1, 512], F32, tag="outc")
            nsb = 4 * ci + 4
            pending = None  # software pipeline: PV(sb-1) emits after ST(sb)
            for sb in range(nsb):
                r = sb - 4 * ci  # >=0 on diagonal s-blocks
                t0 = max(r, 0) * 128
                tw = 512 - t0
                st = stps.tile([128, 512], F32, tag="st")
                nc.tensor.matmul(
                    st[:, :tw],
                    kt_lo[:, sb * 128:(sb + 1) * 128],
                    qk_a[0:64, ci * 512 + t0:(ci + 1) * 512],
                    start=True, stop=True,
                )
                if r >= 0:  # diagonal block: causal mask
                    nc.vector.tensor_tensor(
                        st[:, 0:128], st[:, 0:128], maskd[:], op=ALU.add
                    )
                pt = ptpool.tile([128, 512], BF16, tag="pt")
                nc.scalar.activation(pt[:, :tw], st[:, :tw], AF.Exp, scale=SCALE)
                if pending is not None:
                    nc.tensor.matmul(*pending[0], **pending[1])
                pending = (
                    (out_pc[:, t0:512], v_nat[:, sb, 0:H + 1], pt[:, :tw]),
                    dict(start=(sb == 0), stop=(sb == nsb - 1)),
                )
            nc.tensor.matmul(*pending[0], **pending[1])

            # normalize + transpose + store this chunk
            o_c = opool.tile([H + 1, 512], F32, tag="osb")
            nc.vector.tensor_copy(o_c[:], out_pc[:])
            for rr in range(4):
                tk = ci * 4 + rr
                fin = fps.tile([128, H + 1], F32, tag="fin")
                nc.tensor.transpose(
                    fin[:],
                    o_c[:, rr * 128:(rr + 1) * 128],
                    ident[0:H + 1, 0:H + 1],
                )
                rcp = fpool.tile([128, 1], F32, tag="rcp")
                nc.vector.reciprocal(rcp[:], fin[:, H:H + 1])
                nc.vector.tensor_scalar_mul(
                    o_out[:, tk, :], fin[:, 0:H], rcp[:]
                )
            nc.sync.dma_start(
                outR[:, ci * 4:(ci + 1) * 4, :],
                o_out[:, ci * 4:(ci + 1) * 4, :],
            )

    if DEBUG_TAPS:
        dqk = nc.dram_tensor("dbg_qka", [128, T], BF16,
                             kind="ExternalOutput").ap()
        nc.sync.dma_start(dqk[:], qk_a[:])
        dkt = nc.dram_tensor("dbg_ktlo", [64, T], BF16,
                             kind="ExternalOutput").ap()
        nc.sync.dma_start(dkt[:], kt_lo[:])
        dvn = nc.dram_tensor("dbg_vnat", [128, NTT * VP], BF16,
                             kind="ExternalOutput").ap()
        nc.sync.dma_start(dvn[:], v_nat[:])


def build_nc(reps=1):
    nc = bacc.Bacc("TRN2", target_bir_lowering=False, debug=False)
    xD = nc.dram_tensor("x", [T, C], F32, kind="ExternalInput").ap()
    wqD = nc.dram_tensor("Wq", [C, H], F32, kind="ExternalInput").ap()
    wkD = nc.dram_tensor("Wk", [C, H], F32, kind="ExternalInput").ap()
    wvD = nc.dram_tensor("Wv", [C, H], F32, kind="ExternalInput").ap()
    outD = nc.dram_tensor("out", [T, H], F32, kind="ExternalOutput").ap()

    ALU = mybir.AluOpType

    with tile.TileContext(nc) as tc:
        with (
            tc.tile_pool(name="const", bufs=1) as cpool,
            tc.tile_pool(name="xnat", bufs=2) as xnpool,
            tc.tile_pool(name="xt", bufs=2) as xtpool,
            tc.tile_pool(name="qk", bufs=2) as qkpool,
            tc.tile_pool(name="pt", bufs=3) as ptpool,
            tc.tile_pool(name="osb", bufs=2) as opool,
            tc.tile_pool(name="fin", bufs=2) as fpool,
        ):
            # ---- constants ----
            wqk = cpool.tile([128, NCT, 128], BF16)   # [c_lo, c_hi, (Wq|Wk)]
            nc.gpsimd.dma_start(
                wqk[:, :, 0:H], wqD.rearrange("(k p) h -> p k h", p=128)
            )
            nc.gpsimd.dma_start(
                wqk[:, :, H:128], wkD.rearrange("(k p) h -> p k h", p=128)
            )
            wv = cpool.tile([128, NCT, H], BF16)
            nc.gpsimd.dma_start(wv[:], wvD.rearrange("(k p) h -> p k h", p=128))

            maskd = cpool.tile([128, 128], F32)  # 0 where t>=s else MASKVAL
            nc.gpsimd.memset(maskd[:], 0.0)
            nc.gpsimd.affine_select(
                out=maskd[:], in_=maskd[:],
                compare_op=ALU.is_ge, fill=MASKVAL,
                base=0, pattern=[[1, 128]], channel_multiplier=-1,
            )
            ident = cpool.tile([128, 128], F32)
            make_identity(nc, ident[:])

            consts = (wqk, wv, maskd, ident)
            pools = (xnpool, xtpool, qkpool, ptpool, opool, fpool)
            for _ in range(reps):
                emit_body(nc, tc, xD, outD, consts, pools)

    nc.compile()
    return nc


_NC = None


def kernel(x, Wq, Wk, Wv):
    global _NC
    if _NC is None:
        _NC = build_nc()
    in_maps = [
        {
            "x": np.ascontiguousarray(x[b], dtype=np.float32),
            "Wq": np.ascontiguousarray(Wq, dtype=np.float32),
            "Wk": np.ascontiguousarray(Wk, dtype=np.float32),
            "Wv": np.ascontiguousarray(Wv, dtype=np.float32),
        }
        for b in range(B)
    ]
    res = run_bass_kernel_spmd(_NC, in_maps, core_ids=list(range(B)))
    return np.stack([res.results[b]["out"] for b in range(B)], axis=0)


# revision 18
# speedup vs baseline: 1.2823x; 1.2823x over previous
"""Single-head causal attention on 8 TRN2 NeuronCores.

Problem: x[B=8, T=2048, C=1024], Wq/Wk/Wv[C, H=64] (fp32)
  q = x@Wq; k = x@Wk; v = x@Wv
  wei = softmax(mask(q k^T * C^-0.5)); out = wei @ v       -> [B, T, H]

Sharding: data-parallel over batch, one batch element per core.

Per-core dataflow (all matmuls bf16, fp32 PSUM accumulation):
  1. x [T,C] fp32 --SWDGE cast DMA--> x_nat bf16 [128, 16, 1024] (t-tiles)
  2. xbar transpose DMAs -> xT bf16 [128, 8, 2048]  (C on partitions)
  3. QKV: packed [Wq|Wk] stationary -> psum [qT;kT], Wv -> vT
     scale 1/sqrt(C) is folded into the exp() activation.
  4. S^T tiles = kT.T @ qT  (keys on partitions); causal mask added on
     diagonal tiles; exp on ScalarE (no max subtraction needed: logits
     are O(1) by construction); PV: out_un^T[65, T] accumulates
     [v|ones].T @ exp(S^T)  -- row 64 = sumexp for free.
  5. PE-transpose out_un^T chunks, multiply by 1/sumexp, DMA out.
"""
import sys

sys.path.insert(0, "/opt/trn_rl_repo")

import numpy as np

import concourse.bass as bass
import concourse.mybir as mybir
import concourse.tile as tile
from concourse import bacc
from concourse.bass_utils import run_bass_kernel_spmd
from concourse.masks import make_identity

B, T, C, H = 8, 2048, 1024, 64
NTT = T // 128   # 16 t-tiles
NCT = C // 128   # 8  c-tiles
NCH = T // 512   # 4  t-chunks (moving free dim)
SCALE = float(C) ** -0.5
MASKVAL = -32768.0  # pre-scale additive mask; * SCALE -> -1024 -> exp -> 0
VP = 80          # v_nat per-tile stride: 160B, 32B-aligned for xbar transpose

F32 = mybir.dt.float32
BF16 = mybir.dt.bfloat16

DEBUG_TAPS = False


def emit_body(nc, tc, xD, outD, consts, pools):
    AF = mybir.ActivationFunctionType
    ALU = mybir.AluOpType
    wqk, wv, maskd, ident = consts
    xnpool, xtpool, qkpool, ptpool, opool, fpool = pools

    # ---- load x (cast fp32->bf16) and transpose to xT ----
    x_nat = xnpool.tile([128, NTT, C], BF16, tag="xnat")
    xt = xtpool.tile([128, NCT, T], BF16, tag="xt")
    # 4 big cast-loads: stays within Tile's 4-outstanding-SWDGE window so
    # all loads free-run back-to-back on the DMA engines.
    for g in range(4):
        nc.gpsimd.dma_start(
            x_nat[:, g * 4:(g + 1) * 4, :],
            xD.rearrange("(g p) c -> p g c", p=128)[:, g * 4:(g + 1) * 4, :],
        )
    for tk in range(NTT):
        # one multi-tile xbar transpose per t-tile: [128t, 1024c] ->
        # out[c_lo, c_hi, t] scattered across the 8 c-tiles of xT.
        # All transposes MUST share one HWDGE queue: concurrent xbar
        # transposes on both queues corrupt data (shared xbar state).
        nc.sync.dma_start(
            xt[:, :, tk * 128:(tk + 1) * 128],
            x_nat[:, tk, :],
            transpose=True,
        )

    # ---- QKV projections + attention, pipelined per 512-wide t-chunk ----
    qk_a = qkpool.tile([128, T], BF16, tag="qka")   # rows 0:64 qT, 64:128 kT
    kt_lo = qkpool.tile([64, T], BF16, tag="ktlo")  # kT at partitions 0:64
    vt = qkpool.tile([64, T], BF16, tag="vt")       # vT at partitions 0:64
    v_nat = qkpool.tile([128, NTT, VP], BF16, tag="vnat")  # [s_lo, s_hi, v|1]
    nc.gpsimd.memset(v_nat[:, :, H:H + 1], 1.0)
    o_out = fpool.tile([128, NTT, H], F32, tag="oout")
    outR = outD.rearrange("(g p) h -> p g h", p=128)
    with (
        tc.tile_pool(name="qkps", bufs=2, space="PSUM") as qkps,
        tc.tile_pool(name="vps", bufs=1, space="PSUM") as vps,
        tc.tile_pool(name="ops", bufs=2, space="PSUM") as ops,
        tc.tile_pool(name="stps", bufs=2, space="PSUM") as stps,
        tc.tile_pool(name="fps", bufs=1, space="PSUM") as fps,
    ):
        # PE warm-up: dummy matmuls gated on the first transpose so the
        # HAM clock-gate reaches 8/8 before (and during) real QKV work.
        warm = qkps.tile([128, 512], F32, tag="psqk")
        for _ in range(16):
            nc.tensor.matmul(
                warm[:], wqk[:, 0, :], xt[:, 0:4, 0:128],
                start=True, stop=True,
            )
        for n in range(NCH):
            sl = slice(n * 512, (n + 1) * 512)
            ps_v = vps.tile([64, 512], F32, tag="psv")
            for k in range(NCT):
                nc.tensor.matmul(
                    ps_v[:], wv[:, k, :], xt[:, k, sl],
                    start=(k == 0), stop=(k == NCT - 1),
                )
            nc.vector.tensor_copy(vt[:, sl], ps_v[:])
            # per-chunk xbar transpose: vT[64h, 512s] -> v_nat[s_lo, tk, h]
            nc.sync.dma_start(
                v_nat[:, n * 4:(n + 1) * 4, 0:H], vt[:, sl], transpose=True
            )
            ps_qk = qkps.tile([128, 512], F32, tag="psqk")
            for k in range(NCT):
                nc.tensor.matmul(
                    ps_qk[:], wqk[:, k, :], xt[:, k, sl],
                    start=(k == 0), stop=(k == NCT - 1),
                )
            nc.vector.tensor_copy(qk_a[:, sl], ps_qk[:])
            # kT shifted to partitions 0:64 (stationary operand of S^T).
            # NB must stay on SWDGE: an SBUF->SBUF copy on HWDGE corrupts
            # data when concurrent with xbar transposes (known HW hazard).
            nc.gpsimd.dma_start(kt_lo[:, sl], qk_a[64:128, sl])

        # attention, column-chunk outer: chunk ci only needs QKV chunks
        # <= ci, so early chunks overlap later projections
        for ci in range(NCH):
            out_pc = ops.tile([H + 1, 512], F32, tag="outc")
            nsb = 4 * ci + 4
            pending = None  # software pipeline: PV(sb-1) emits after ST(sb)
            for sb in range(nsb):
                r = sb - 4 * ci  # >=0 on diagonal s-blocks
                t0 = max(r, 0) * 128
                tw = 512 - t0
                st = stps.tile([128, 512], F32, tag="st")
                nc.tensor.matmul(
                    st[:, :tw],
                    kt_lo[:, sb * 128:(sb + 1) * 128],
                    qk_a[0:64, ci * 512 + t0:(ci + 1) * 512],
                    start=True, stop=True,
                )
                if r >= 0:  # diagonal block: causal mask
                    nc.vector.tensor_tensor(
                        st[:, 0:128], st[:, 0:128], maskd[:], op=ALU.add
                    )
                pt = ptpool.tile([128, 512], BF16, tag="pt")
                nc.scalar.activation(pt[:, :tw], st[:, :tw], AF.Exp, scale=SCALE)
                if pending is not None:
                    nc.tensor.matmul(*pending[0], **pending[1])
                pending = (
                    (out_pc[:, t0:512], v_nat[:, sb, 0:H + 1], pt[:, :tw]),
                    dict(start=(sb == 0), stop=(sb == nsb - 1)),
                )
            nc.tensor.matmul(*pending[0], **pending[1])

            # normalize + transpose + store this chunk
            o_c = opool.tile([H + 1, 512], F32, tag="osb")
            nc.vector.tensor_copy(o_c[:], out_pc[:])
            for rr in range(4):
                tk = ci * 4 + rr
                fin = fps.tile([128, H + 1], F32, tag="fin")
                nc.tensor.transpose(
                    fin[:],
                    o_c[:, rr * 128:(rr + 1) * 128],
                    ident[0:H + 1, 0:H + 1],
                )
                rcp = fpool.tile([128, 1], F32, tag="rcp")
                nc.vector.reciprocal(rcp[:], fin[:, H:H + 1])
                nc.vector.tensor_scalar_mul(
                    o_out[:, tk, :], fin[:, 0:H], rcp[:]
                )
            nc.sync.dma_start(
                outR[:, ci * 4:(ci + 1) * 4, :],
                o_out[:, ci * 4:(ci + 1) * 4, :],
            )

    if DEBUG_TAPS:
        dqk = nc.dram_tensor("dbg_qka", [128, T], BF16,
                             kind="ExternalOutput").ap()
        nc.sync.dma_start(dqk[:], qk_a[:])
        dkt = nc.dram_tensor("dbg_ktlo", [64, T], BF16,
                             kind="ExternalOutput").ap()
        nc.sync.dma_start(dkt[:], kt_lo[:])
        dvn = nc.dram_tensor("dbg_vnat", [128, NTT * VP], BF16,
                             kind="ExternalOutput").ap()
        nc.sync.dma_start(dvn[:], v_nat[:])


def build_nc(reps=1):
    nc = bacc.Bacc("TRN2", target_bir_lowering=False, debug=False)
    xD = nc.dram_tensor("x", [T, C], F32, kind="ExternalInput").ap()
    wqD = nc.dram_tensor("Wq", [C, H], F32, kind="ExternalInput").ap()
    wkD = nc.dram_tensor("Wk", [C, H], F32, kind="ExternalInput").ap()
    wvD = nc.dram_tensor("Wv", [C, H], F32, kind="ExternalInput").ap()
    outD = nc.dram_tensor("out", [T, H], F32, kind="ExternalOutput").ap()

    ALU = mybir.AluOpType

    with tile.TileContext(nc) as tc:
        with (
            tc.tile_pool(name="const", bufs=1) as cpool,
            tc.tile_pool(name="xnat", bufs=2) as xnpool,
            tc.tile_pool(name="xt", bufs=2) as xtpool,
            tc.tile_pool(name="qk", bufs=2) as qkpool,
            tc.tile_pool(name="pt", bufs=3) as ptpool,
            tc.tile_pool(name="osb", bufs=2) as opool,
            tc.tile_pool(name="fin", bufs=2) as fpool,
        ):
            # ---- constants ----
            wqk = cpool.tile([128, NCT, 128], BF16)   # [c_lo, c_hi, (Wq|Wk)]
            nc.gpsimd.dma_start(
                wqk[:, :, 0:H], wqD.rearrange("(k p) h -> p k h", p=128)
            )
            nc.gpsimd.dma_start(
                wqk[:, :, H:128], wkD.rearrange("(k p) h -> p k h", p=128)
            )
            wv = cpool.tile([128, NCT, H], BF16)
            nc.gpsimd.dma_start(wv[:], wvD.rearrange("(k p) h -> p k h", p=128))

            maskd = cpool.tile([128, 128], F32)  # 0 where t>=s else MASKVAL
            nc.gpsimd.memset(maskd[:], 0.0)
            nc.gpsimd.affine_select(
                out=maskd[:], in_=maskd[:],
                compare_op=ALU.is_ge, fill=MASKVAL,
                base=0, pattern=[[1, 128]], channel_multiplier=-1,
            )
            ident = cpool.tile([128, 128], F32)
            make_identity(nc, ident[:])

            consts = (wqk, wv, maskd, ident)
            pools = (xnpool, xtpool, qkpool, ptpool, opool, fpool)
            for _ in range(reps):
                emit_body(nc, tc, xD, outD, consts, pools)

    nc.compile()
    return nc


_NC = None


def kernel(x, Wq, Wk, Wv):
    global _NC
    if _NC is None:
        _NC = build_nc()
    in_maps = [
        {
            "x": np.ascontiguousarray(x[b], dtype=np.float32),
            "Wq": np.ascontiguousarray(Wq, dtype=np.float32),
            "Wk": np.ascontiguousarray(Wk, dtype=np.float32),
            "Wv": np.ascontiguousarray(Wv, dtype=np.float32),
        }
        for b in range(B)
    ]
    res = run_bass_kernel_spmd(_NC, in_maps, core_ids=list(range(B)))
    return np.stack([res.results[b]["out"] for b in range(B)], axis=0)


# revision 19
# speedup vs baseline: 1.4126x; 1.1016x over previous
"""Single-head causal attention on 8 TRN2 NeuronCores.

Problem: x[B=8, T=2048, C=1024], Wq/Wk/Wv[C, H=64] (fp32)
  q = x@Wq; k = x@Wk; v = x@Wv
  wei = softmax(mask(q k^T * C^-0.5)); out = wei @ v       -> [B, T, H]

Sharding: data-parallel over batch, one batch element per core.

Per-core dataflow (all matmuls bf16, fp32 PSUM accumulation):
  1. x [T,C] fp32 --SWDGE cast DMA--> x_nat bf16 [128, 16, 1024] (t-tiles)
  2. xbar transpose DMAs -> xT bf16 [128, 8, 2048]  (C on partitions)
  3. QKV: packed [Wq|Wk] stationary -> psum [qT;kT], Wv -> vT
     scale 1/sqrt(C) is folded into the exp() activation.
  4. S^T tiles = kT.T @ qT  (keys on partitions); causal mask added on
     diagonal tiles; exp on ScalarE (no max subtraction needed: logits
     are O(1) by construction); PV: out_un^T[65, T] accumulates
     [v|ones].T @ exp(S^T)  -- row 64 = sumexp for free.
  5. PE-transpose out_un^T chunks, multiply by 1/sumexp, DMA out.
"""
import sys

sys.path.insert(0, "/opt/trn_rl_repo")

import numpy as np

import concourse.bass as bass
import concourse.mybir as mybir
import concourse.tile as tile
from concourse import bacc
from concourse.bass_utils import run_bass_kernel_spmd
from concourse.masks import make_identity

B, T, C, H = 8, 2048, 1024, 64
NTT = T // 128   # 16 t-tiles
NCT = C // 128   # 8  c-tiles
NCH = T // 512   # 4  t-chunks (moving free dim)
SCALE = float(C) ** -0.5
MASKVAL = -32768.0  # pre-scale additive mask; * SCALE -> -1024 -> exp -> 0
VP = 80          # v_nat per-tile stride: 160B, 32B-aligned for xbar transpose

F32 = mybir.dt.float32
BF16 = mybir.dt.bfloat16

DEBUG_TAPS = False


def emit_body(nc, tc, xD, outD, consts, pools):
    AF = mybir.ActivationFunctionType
    ALU = mybir.AluOpType
    wqk, wv, maskd, ident = consts
    xnpool, xtpool, qkpool, ptpool, opool, fpool = pools

    # ---- load x (cast fp32->bf16) and transpose to xT ----
    x_nat = xnpool.tile([128, NTT, C], BF16, tag="xnat")
    xt = xtpool.tile([128, NCT, T], BF16, tag="xt")
    # 2 big cast-loads: Tile gates SWDGE DMAs beyond a small in-flight
    # window on unrelated consumers, so keep the count minimal.
    for g in range(2):
        nc.gpsimd.dma_start(
            x_nat[:, g * 8:(g + 1) * 8, :],
            xD.rearrange("(g p) c -> p g c", p=128)[:, g * 8:(g + 1) * 8, :],
        )
    for tk in range(NTT):
        # one multi-tile xbar transpose per t-tile: [128t, 1024c] ->
        # out[c_lo, c_hi, t] scattered across the 8 c-tiles of xT.
        # All transposes MUST share one HWDGE queue: concurrent xbar
        # transposes on both queues corrupt data (shared xbar state).
        nc.sync.dma_start(
            xt[:, :, tk * 128:(tk + 1) * 128],
            x_nat[:, tk, :],
            transpose=True,
        )

    # ---- QKV projections + attention, pipelined per 512-wide t-chunk ----
    qk_a = qkpool.tile([128, T], BF16, tag="qka")   # rows 0:64 qT, 64:128 kT
    kt_lo = qkpool.tile([64, T], BF16, tag="ktlo")  # kT at partitions 0:64
    vt = qkpool.tile([64, T], BF16, tag="vt")       # vT at partitions 0:64
    v_nat = qkpool.tile([128, NTT, VP], BF16, tag="vnat")  # [s_lo, s_hi, v|1]
    nc.gpsimd.memset(v_nat[:, :, H:H + 1], 1.0)
    o_out = fpool.tile([128, NTT, H], F32, tag="oout")
    outR = outD.rearrange("(g p) h -> p g h", p=128)
    with (
        tc.tile_pool(name="qkps", bufs=2, space="PSUM") as qkps,
        tc.tile_pool(name="vps", bufs=1, space="PSUM") as vps,
        tc.tile_pool(name="ops", bufs=2, space="PSUM") as ops,
        tc.tile_pool(name="stps", bufs=2, space="PSUM") as stps,
        tc.tile_pool(name="fps", bufs=1, space="PSUM") as fps,
    ):
        # PE warm-up: dummy matmuls gated on the first transpose so the
        # HAM clock-gate reaches 8/8 before (and during) real QKV work.
        warm = qkps.tile([128, 512], F32, tag="psqk")
        for _ in range(16):
            nc.tensor.matmul(
                warm[:], wqk[:, 0, :], xt[:, 0:4, 0:128],
                start=True, stop=True,
            )
        for n in range(NCH):
            sl = slice(n * 512, (n + 1) * 512)
            ps_v = vps.tile([64, 512], F32, tag="psv")
            for k in range(NCT):
                nc.tensor.matmul(
                    ps_v[:], wv[:, k, :], xt[:, k, sl],
                    start=(k == 0), stop=(k == NCT - 1),
                )
            nc.vector.tensor_copy(vt[:, sl], ps_v[:])
            # per-chunk xbar transpose: vT[64h, 512s] -> v_nat[s_lo, tk, h]
            nc.sync.dma_start(
                v_nat[:, n * 4:(n + 1) * 4, 0:H], vt[:, sl], transpose=True
            )
            ps_qk = qkps.tile([128, 512], F32, tag="psqk")
            for k in range(NCT):
                nc.tensor.matmul(
                    ps_qk[:], wqk[:, k, :], xt[:, k, sl],
                    start=(k == 0), stop=(k == NCT - 1),
                )
            nc.vector.tensor_copy(qk_a[:, sl], ps_qk[:])
            # kT shifted to partitions 0:64 (stationary operand of S^T).
            # NB must stay on SWDGE: an SBUF->SBUF copy on HWDGE corrupts
            # data when concurrent with xbar transposes (known HW hazard).
            nc.gpsimd.dma_start(kt_lo[:, sl], qk_a[64:128, sl])

        # attention, column-chunk outer: chunk ci only needs QKV chunks
        # <= ci, so early chunks overlap later projections
        for ci in range(NCH):
            out_pc = ops.tile([H + 1, 512], F32, tag="outc")
            nsb = 4 * ci + 4
            pending = None  # software pipeline: PV(sb-1) emits after ST(sb)
            for sb in range(nsb):
                r = sb - 4 * ci  # >=0 on diagonal s-blocks
                t0 = max(r, 0) * 128
                tw = 512 - t0
                st = stps.tile([128, 512], F32, tag="st")
                nc.tensor.matmul(
                    st[:, :tw],
                    kt_lo[:, sb * 128:(sb + 1) * 128],
                    qk_a[0:64, ci * 512 + t0:(ci + 1) * 512],
                    start=True, stop=True,
                )
                if r >= 0:  # diagonal block: causal mask
                    nc.vector.tensor_tensor(
                        st[:, 0:128], st[:, 0:128], maskd[:], op=ALU.add
                    )
                pt = ptpool.tile([128, 512], BF16, tag="pt")
                nc.scalar.activation(pt[:, :tw], st[:, :tw], AF.Exp, scale=SCALE)
                if pending is not None:
                    nc.tensor.matmul(*pending[0], **pending[1])
                pending = (
                    (out_pc[:, t0:512], v_nat[:, sb, 0:H + 1], pt[:, :tw]),
                    dict(start=(sb == 0), stop=(sb == nsb - 1)),
                )
            nc.tensor.matmul(*pending[0], **pending[1])

            # normalize + transpose + store this chunk
            o_c = opool.tile([H + 1, 512], F32, tag="osb")
            nc.vector.tensor_copy(o_c[:], out_pc[:])
            for rr in range(4):
                tk = ci * 4 + rr
                fin = fps.tile([128, H + 1], F32, tag="fin")
                nc.tensor.transpose(
                    fin[:],
                    o_c[:, rr * 128:(rr + 1) * 128],
                    ident[0:H + 1, 0:H + 1],
                )
                rcp = fpool.tile([128, 1], F32, tag="rcp")
                nc.vector.reciprocal(rcp[:], fin[:, H:H + 1])
                nc.vector.tensor_scalar_mul(
                    o_out[:, tk, :], fin[:, 0:H], rcp[:]
                )
            nc.sync.dma_start(
                outR[:, ci * 4:(ci + 1) * 4, :],
                o_out[:, ci * 4:(ci + 1) * 4, :],
            )

    if DEBUG_TAPS:
        dqk = nc.dram_tensor("dbg_qka", [128, T], BF16,
                             kind="ExternalOutput").ap()
        nc.sync.dma_start(dqk[:], qk_a[:])
        dkt = nc.dram_tensor("dbg_ktlo", [64, T], BF16,
                             kind="ExternalOutput").ap()
        nc.sync.dma_start(dkt[:], kt_lo[:])
        dvn = nc.dram_tensor("dbg_vnat", [128, NTT * VP], BF16,
                             kind="ExternalOutput").ap()
        nc.sync.dma_start(dvn[:], v_nat[:])


def build_nc(reps=1):
    nc = bacc.Bacc("TRN2", target_bir_lowering=False, debug=False)
    xD = nc.dram_tensor("x", [T, C], F32, kind="ExternalInput").ap()
    wqD = nc.dram_tensor("Wq", [C, H], F32, kind="ExternalInput").ap()
    wkD = nc.dram_tensor("Wk", [C, H], F32, kind="ExternalInput").ap()
    wvD = nc.dram_tensor("Wv", [C, H], F32, kind="ExternalInput").ap()
    outD = nc.dram_tensor("out", [T, H], F32, kind="ExternalOutput").ap()

    ALU = mybir.AluOpType

    with tile.TileContext(nc) as tc:
        with (
            tc.tile_pool(name="const", bufs=1) as cpool,
            tc.tile_pool(name="xnat", bufs=2) as xnpool,
            tc.tile_pool(name="xt", bufs=2) as xtpool,
            tc.tile_pool(name="qk", bufs=2) as qkpool,
            tc.tile_pool(name="pt", bufs=3) as ptpool,
            tc.tile_pool(name="osb", bufs=2) as opool,
            tc.tile_pool(name="fin", bufs=2) as fpool,
        ):
            # ---- constants ----
            wqk = cpool.tile([128, NCT, 128], BF16)   # [c_lo, c_hi, (Wq|Wk)]
            nc.gpsimd.dma_start(
                wqk[:, :, 0:H], wqD.rearrange("(k p) h -> p k h", p=128)
            )
            nc.gpsimd.dma_start(
                wqk[:, :, H:128], wkD.rearrange("(k p) h -> p k h", p=128)
            )
            wv = cpool.tile([128, NCT, H], BF16)
            nc.gpsimd.dma_start(wv[:], wvD.rearrange("(k p) h -> p k h", p=128))

            maskd = cpool.tile([128, 128], F32)  # 0 where t>=s else MASKVAL
            nc.gpsimd.memset(maskd[:], 0.0)
            nc.gpsimd.affine_select(
                out=maskd[:], in_=maskd[:],
                compare_op=ALU.is_ge, fill=MASKVAL,
                base=0, pattern=[[1, 128]], channel_multiplier=-1,
            )
            ident = cpool.tile([128, 128], F32)
            make_identity(nc, ident[:])

            consts = (wqk, wv, maskd, ident)
            pools = (xnpool, xtpool, qkpool, ptpool, opool, fpool)
            for _ in range(reps):
                emit_body(nc, tc, xD, outD, consts, pools)

    nc.compile()
    return nc


_NC = None


def kernel(x, Wq, Wk, Wv):
    global _NC
    if _NC is None:
        _NC = build_nc()
    in_maps = [
        {
            "x": np.ascontiguousarray(x[b], dtype=np.float32),
            "Wq": np.ascontiguousarray(Wq, dtype=np.float32),
            "Wk": np.ascontiguousarray(Wk, dtype=np.float32),
            "Wv": np.ascontiguousarray(Wv, dtype=np.float32),
        }
        for b in range(B)
    ]
    res = run_bass_kernel_spmd(_NC, in_maps, core_ids=list(range(B)))
    return np.stack([res.results[b]["out"] for b in range(B)], axis=0)
